# revision 7
# baseline (speedup 1.0000x reference)
"""Trainium2 Bass kernel for nn_Net_67954972557347 (dense_mlp).

Network: a1 = lrelu(a@Wa+ba) [B,68]; b1 = lrelu(b@Wb+bb) [B,68];
c = [a1|b1|meta] [B,140]; then 10 lrelu'd dense layers
(140->34->34->20->20->20->20->20->5->2->1), lrelu slope 0.01.

Pure data parallel over 8 cores (32768 rows each). On-device the batch
streams feature-major in 512-column chunks, all-fp16 datapath. Three PSUM
banks and 6 matmuls per step cover all 12 layers, with the deep chain
interleaved across the banks so each bank drains PSUM->SBUF in ONE
contiguous window:

  bank A [126] = [a1(68), meta(4), d1(34), d3(20)]  <- MM(T2 DMA) + MM(T_C)
  bank B [94]  = [b1(68), d5(20), d7(5), y(1)]      <- MM(T1 DMA) + MM(T_C)
  bank C [96]  = [d0(34), d2(20), d4(20), d6(20), d8(2)] <- MM(T_A)+MM(T_B)

A and B drain on the scalar engine (Prelu + per-partition bias vector);
C drains on the vector engine (tensor_scalar_add bias + lrelu via
scalar_tensor_tensor). Every consumer matmul reads the drain tile from
step t-2 (not t-1), so drain latency amortizes over two chunk periods and
the PE stays continuously busy (full p-state). A chunk entering at step s
leaves as y (DMA'd from T_B row 93 on gpsimd) at step s+20.
"""

import os
import sys

import numpy as np

for _p in ("/opt/trn_rl_repo", "/root/.axon_site/_ro/trn_rl_repo"):
    if os.path.isdir(_p) and _p not in sys.path:
        sys.path.append(_p)

import concourse.bass as bass
import concourse.mybir as mybir
import concourse.tile as tile
from concourse import bacc
from concourse.bass_utils import run_bass_kernel_spmd
from bass_rust import add_dep_helper

F32 = mybir.dt.float32
F16 = mybir.dt.float16
ALU = mybir.AluOpType
PRELU = mybir.ActivationFunctionType.Prelu
NPF16 = np.float16

B_FULL = 262144
N_CORES = 8
B_CORE = B_FULL // N_CORES          # 32768
N = 512                              # columns per chunk (fp32 PSUM bank cap)
PIPE = 15                            # pipeline latency in steps
ALPHA = 0.01                         # leaky-relu slope

# weight tile column blocks
CA_IN, CA_CH, CB_IN, CB_CH, CC_A, CC_B = 0, 128, 256, 384, 512, 640
WT_COLS = 768
M_A, M_B, M_C = 126, 94, 96         # bank partition heights
K_A_IN, K_A_CH = 49, 54
K_B_IN, K_B_CH = 102, 96
K_C_A, K_C_B = 126, 93


def _ilrelu(x):
    """Inverse of leaky-relu (slope 0.01)."""
    return np.where(x > 0, x, x * (1.0 / ALPHA)).astype(np.float32)


def _pack_weights(Wa, ba, Wb, bb, Ws, Bs):
    """Packed [128, WT_COLS] fp16 weight tile + [128, 3] f32 biases."""
    W0, W1, W2, W3, W4, W5, W6, W7, W8, W9 = Ws
    B0, B1, B2, B3, B4, B5, B6, B7, B8, B9 = Bs
    wt = np.zeros((128, WT_COLS), np.float32)
    # (T2, A): rows [a(45); meta(4)] -> cols [a1 0:68, meta 68:72]
    wt[0:45, CA_IN + 0:CA_IN + 68] = Wa
    wt[45:49, CA_IN + 68:CA_IN + 72] = np.eye(4, dtype=np.float32)
    # (T_C, A): rows [d0 0:34, d2 34:54] -> cols [d1 72:106, d3 106:126]
    wt[0:34, CA_CH + 72:CA_CH + 106] = W1
    wt[34:54, CA_CH + 106:CA_CH + 126] = W3
    # (T1, B): rows b 0:102 -> cols b1 0:68
    wt[0:102, CB_IN + 0:CB_IN + 68] = Wb
    # (T_C, B): rows [d4 54:74, d6 74:94, d8 94:96]
    #        -> cols [d5 68:88, d7 88:93, y 93:94]
    wt[54:74, CB_CH + 68:CB_CH + 88] = W5
    wt[74:94, CB_CH + 88:CB_CH + 93] = W7
    wt[94:96, CB_CH + 93:CB_CH + 94] = W9
    # (T_A, C): rows [a1 0:68, meta 68:72, d1 72:106, d3 106:126]
    #        -> cols [d0 0:34, d2 34:54, d4 54:74]
    wt[0:68, CC_A + 0:CC_A + 34] = W0[0:68]
    wt[68:72, CC_A + 0:CC_A + 34] = W0[136:140]
    wt[72:106, CC_A + 34:CC_A + 54] = W2
    wt[106:126, CC_A + 54:CC_A + 74] = W4
    # (T_B, C): rows [b1 0:68, d5 68:88, d7 88:93]
    #        -> cols [d0 0:34, d6 74:94, d8 94:96]
    wt[0:68, CC_B + 0:CC_B + 34] = W0[68:136]
    wt[68:88, CC_B + 74:CC_B + 94] = W6
    wt[88:93, CC_B + 94:CC_B + 96] = W8

    aux = np.zeros((128, 3), np.float32)
    # col 0: bank-A drain bias [ba, 0(meta), B1, B3]
    aux[0:68, 0] = ba
    aux[72:106, 0] = B1
    aux[106:126, 0] = B3
    # col 1: bank-C drain bias [B0, B2, B4, B6, B8]
    aux[0:34, 1] = B0
    aux[34:54, 1] = B2
    aux[54:74, 1] = B4
    aux[74:94, 1] = B6
    aux[94:96, 1] = B8
    # col 2: bank-B drain bias [bb, B5, B7, B9]
    aux[0:68, 2] = bb
    aux[68:88, 2] = B5
    aux[88:93, 2] = B7
    aux[93:94, 2] = B9
    return wt.astype(NPF16), aux


def _pack_core_inputs(a, b, meta, n_chunks):
    """One core's shard -> (t1 [102,bc], t2 [49,bc]) fp16 streams."""
    bc = n_chunks * N
    t1 = np.ascontiguousarray(b[:bc].T).astype(NPF16)
    t2 = np.empty((49, bc), np.float32)
    t2[0:45] = a[:bc].T
    t2[45:49] = _ilrelu(meta[:bc].T)
    return t1, t2.astype(NPF16)


def build_bass(n_chunks):
    nc = bacc.Bacc(None, target_bir_lowering=False, debug=False)
    n_steps = n_chunks + PIPE

    t1_d = nc.dram_tensor("t1", [K_B_IN, n_chunks * N], F16,
                          kind="ExternalInput")
    t2_d = nc.dram_tensor("t2", [K_A_IN, n_chunks * N], F16,
                          kind="ExternalInput")
    wt_d = nc.dram_tensor("wt", [128, WT_COLS], F16, kind="ExternalInput")
    aux_d = nc.dram_tensor("aux", [128, 3], F32, kind="ExternalInput")
    y_d = nc.dram_tensor("y", [1, n_chunks * N], F16, kind="ExternalOutput")

    with tile.TileContext(nc) as tc:
        with (
            tc.tile_pool(name="const", bufs=1) as constp,
            tc.tile_pool(name="t1p", bufs=6) as t1p,
            tc.tile_pool(name="t2p", bufs=6) as t2p,
            tc.tile_pool(name="tap", bufs=3) as tap,
            tc.tile_pool(name="tbp", bufs=3) as tbp,
            tc.tile_pool(name="tcp", bufs=3) as tcp,
            tc.tile_pool(name="ctmpp", bufs=3) as ctmpp,
            tc.tile_pool(name="ps", bufs=1, space=bass.MemorySpace.PSUM) as ps,
        ):
            wt = constp.tile([128, WT_COLS], F16, tag="wt")
            aux = constp.tile([128, 3], F32, tag="aux")
            nc.sync.dma_start(wt[:], wt_d[:])
            nc.sync.dma_start(aux[:], aux_d[:])

            def w(c0, k, m):
                return wt[0:k, c0:c0 + m]

            def chain(a, b):
                add_dep_helper(b.ins, a.ins, sync=False,
                               reason="psum acc order")

            t1s, t2s, tas, tbs, tcs = {}, {}, {}, {}, {}
            mm = nc.tensor.matmul
            nc_ = n_chunks

            # PSUM placement: sibling banks (2k, 2k+1) must not take
            # step-alternating PE writes, so each tag's parity buffers go
            # to different pairs: (pa0,pb0) (pa1,pb1) (pc0,z) (pc1,z)
            pa2 = [ps.tile([M_A, N], F32, tag="pa0", name="pa0"), None]
            pb2 = [ps.tile([M_B, N], F32, tag="pb0", name="pb0"), None]
            pa2[1] = ps.tile([M_A, N], F32, tag="pa1", name="pa1")
            pb2[1] = ps.tile([M_B, N], F32, tag="pb1", name="pb1")
            pc2 = [ps.tile([M_C, N], F32, tag="pc0", name="pc0"), None]
            zz0 = ps.tile([128, 8], F32, tag="zz0", name="zz0")
            pc2[1] = ps.tile([M_C, N], F32, tag="pc1", name="pc1")
            zz1 = ps.tile([128, 8], F32, tag="zz1", name="zz1")

            def dma_in(t):
                if t >= nc_:
                    return
                t1s[t] = t1p.tile([K_B_IN, N], F16, tag="t1",
                                  name=f"t1_{t}")
                nc.sync.dma_start(t1s[t][:], t1_d[:, t * N:(t + 1) * N])
                t2s[t] = t2p.tile([K_A_IN, N], F16, tag="t2",
                                  name=f"t2_{t}")
                nc.sync.dma_start(t2s[t][:], t2_d[:, t * N:(t + 1) * N])

            for t in range(3):
                dma_in(t)

            for t in range(n_steps):
                has_ain = t < nc_
                has_ach = 3 <= t <= nc_ + 5
                has_bin = t < nc_
                has_bch = 9 <= t <= nc_ + 14
                has_ca = 1 <= t <= nc_ + 6
                has_cb = 1 <= t <= nc_ + 12

                dma_in(t + 3)

                pa = pa2[t % 2] if (has_ain or has_ach) else None
                pb = pb2[t % 2] if (has_bin or has_bch) else None
                pc = pc2[t % 2] if (has_ca or has_cb) else None

                if has_bin:
                    i1 = mm(pb[0:M_B], w(CB_IN, K_B_IN, M_B),
                            t1s[t][0:K_B_IN],
                            start=True, stop=not has_bch,
                            tile_position=(0, 0))
                if has_bch:
                    i2 = mm(pb[0:M_B], w(CB_CH, K_B_CH, M_B),
                            tcs[t - 2][0:K_B_CH],
                            start=not has_bin, stop=True,
                            tile_position=(0, 0))
                    if has_bin:
                        chain(i1, i2)

                if has_ain:
                    i1 = mm(pa[0:M_A], w(CA_IN, K_A_IN, M_A),
                            t2s[t][0:K_A_IN],
                            start=True, stop=not has_ach,
                            tile_position=(0, 0))
                if has_ach:
                    i2 = mm(pa[0:M_A], w(CA_CH, K_A_CH, M_A),
                            tcs[t - 2][0:K_A_CH],
                            start=not has_ain, stop=True,
                            tile_position=(0, 0))
                    if has_ain:
                        chain(i1, i2)

                if has_ca:
                    i1 = mm(pc[0:M_C], w(CC_A, K_C_A, M_C),
                            tas[t - 1][0:K_C_A],
                            start=True, stop=not has_cb,
                            tile_position=(0, 0))
                if has_cb:
                    i2 = mm(pc[0:M_C], w(CC_B, K_C_B, M_C),
                            tbs[t - 1][0:K_C_B],
                            start=not has_ca, stop=True,
                            tile_position=(0, 0))
                    if has_ca:
                        chain(i1, i2)

                # drains
                if pa is not None:
                    tas[t] = tap.tile([M_A, N], F16, tag="ta",
                                      name=f"ta_{t}")
                    nc.scalar.activation(tas[t][0:M_A], pa[0:M_A], PRELU,
                                         bias=aux[0:M_A, 0:1], alpha=ALPHA)
                if pb is not None:
                    tbs[t] = tbp.tile([M_B, N], F16, tag="tb",
                                      name=f"tb_{t}")
                    nc.scalar.activation(tbs[t][0:M_B], pb[0:M_B], PRELU,
                                         bias=aux[0:M_B, 2:3], alpha=ALPHA)
                if pc is not None:
                    ctmp = ctmpp.tile([M_C, N], F16, tag="ct",
                                      name=f"ct_{t}")
                    nc.vector.tensor_scalar_add(ctmp[0:M_C], pc[0:M_C],
                                                aux[0:M_C, 1:2])
                    tcs[t] = tcp.tile([M_C, N], F16, tag="tc",
                                      name=f"tc_{t}")
                    nc.vector.scalar_tensor_tensor(
                        tcs[t][0:M_C], ctmp[0:M_C], ALPHA, ctmp[0:M_C],
                        ALU.mult, ALU.max)

                # y out: chunk t-PIPE sits in T_B(t) row 93
                if t >= PIPE:
                    c = t - PIPE
                    nc.gpsimd.dma_start(y_d[:, c * N:(c + 1) * N],
                                        tbs[t][93:94])

    nc.compile()
    return nc


_NC_CACHE = {}


def _get_nc(n_chunks):
    if n_chunks not in _NC_CACHE:
        _NC_CACHE[n_chunks] = build_bass(n_chunks)
    return _NC_CACHE[n_chunks]


def run_cores(inputs, n_chunks, cores, trace=False, trace_kwargs=None):
    a = np.asarray(inputs["a"], np.float32)
    b = np.asarray(inputs["b"], np.float32)
    meta = np.asarray(inputs["meta"], np.float32)
    Ws = [np.asarray(inputs[f"W{i}"], np.float32) for i in range(10)]
    Bs = [np.asarray(inputs[f"B{i}"], np.float32) for i in range(10)]
    wt, aux = _pack_weights(np.asarray(inputs["Wa"], np.float32),
                            np.asarray(inputs["ba"], np.float32),
                            np.asarray(inputs["Wb"], np.float32),
                            np.asarray(inputs["bb"], np.float32), Ws, Bs)
    in_maps = []
    for r in cores:
        sl = slice(r * B_CORE, r * B_CORE + n_chunks * N)
        t1, t2 = _pack_core_inputs(a[sl], b[sl], meta[sl], n_chunks)
        in_maps.append({"t1": t1, "t2": t2, "wt": wt, "aux": aux})
    nc = _get_nc(n_chunks)
    kw = dict(trace=trace)
    if trace_kwargs:
        kw.update(trace_kwargs)
    res = run_bass_kernel_spmd(nc, in_maps, list(range(len(cores))), **kw)
    return [res.results[i]["y"] for i in range(len(cores))], res


def kernel(**inputs):
    n_chunks = B_CORE // N
    ys, _ = run_cores(inputs, n_chunks, list(range(N_CORES)))
    out = np.empty((B_FULL, 1), np.float32)
    for r in range(N_CORES):
        out[r * B_CORE:(r + 1) * B_CORE, 0] = \
            np.asarray(ys[r][0], np.float32)
    return out
